# revision 18
# baseline (speedup 1.0000x reference)
"""Encoder layer (pre-norm attention + MLP) on 8 Trainium2 cores.

Sharding: core = (batch b in 0..3, half hf in 0..1). Each core receives the
full 2048-token sequence of batch b, transposed to [E, S] and rolled so the
core's own 1024 tokens are columns 0:1024 (attention and LN are invariant to
key order, so rolling keeps the program identical across cores). The core
computes K/V over the full sequence and everything else only for its own
tokens. No collectives; the host reassembles the 8 shards.

v2 layout: everything stays in SBUF between stages (k/q/v, h) — no DRAM
round trips. LN1+QKV run as one chunk-pipelined phase (stats -> z1 -> QKV
matmuls per 512-token chunk) so the tensor engine never idles waiting for
the full-sequence layernorm. Attention score matmuls (contraction = 64) are
row-packed in head pairs onto the two 64-row halves of the PE array
(tile_position (0,0)/(64,0)) for ~2x throughput, and exp is batched over
2-bank PSUM tiles. Softmax denominators use the fast approximate DVE
reciprocal. The MLP runs from SBUF with bf16 weights; the final residual
add + fc2 bias happen on-device, so the kernel emits one output tensor.
"""

import numpy as np
import ml_dtypes
from contextlib import ExitStack

import concourse.bacc as bacc
import concourse.mybir as mybir
import concourse.tile as tile
from concourse.bass_utils import run_bass_kernel_spmd

F32 = mybir.dt.float32
F32R = mybir.dt.float32r
BF16 = mybir.dt.bfloat16
AF = mybir.ActivationFunctionType
OP = mybir.AluOpType

B, S, E, H, D, FF = 4, 2048, 1024, 16, 64, 4096
TOWN = 1024  # tokens owned per core
ET = E // 128  # 8
FT = FF // 128  # 32
NT = S // 128  # 16 token tiles (full seq)
NCORES = 8
EPS = 1e-6
CA = 512  # token chunk for the fused LN1+QKV phase
NCH = S // CA  # 4


def _build(debug=False):
    nc = bacc.Bacc()

    x_t = nc.dram_tensor("x_t", [E, S], F32R, kind="ExternalInput")
    # streamed per-out-tile qkv weights: [out_tile, 128(p), in_tile, 128]
    wq_t = nc.dram_tensor("wq_t", [ET, 128, ET, 128], BF16, kind="ExternalInput")
    wk_t = nc.dram_tensor("wk_t", [ET, 128, ET, 128], BF16, kind="ExternalInput")
    # resident v weights, partition-major: [128(p), half, in_tile, 512]
    wv_t = nc.dram_tensor("wv_t", [128, 2, ET, 512], BF16, kind="ExternalInput")
    qb = nc.dram_tensor("qb", [128, ET], F32, kind="ExternalInput")
    kb = nc.dram_tensor("kb", [128, ET], F32, kind="ExternalInput")
    vb = nc.dram_tensor("vb", [E], F32R, kind="ExternalInput")
    # streamed weights: per-out-tile blocks [out_tile, 128(p), in_tile, k]
    wout_t = nc.dram_tensor("wout_t", [ET, 128, ET, 128], BF16,
                            kind="ExternalInput")
    ob = nc.dram_tensor("ob", [128, ET], F32, kind="ExternalInput")
    wfc1_t = nc.dram_tensor("wfc1_t", [FT, 128, ET, 128], F32R,
                            kind="ExternalInput")
    f1b = nc.dram_tensor("f1b", [128, FT], F32, kind="ExternalInput")
    wfc2_t = nc.dram_tensor("wfc2_t", [ET, 128, FT, 128], F32R,
                            kind="ExternalInput")
    f2b = nc.dram_tensor("f2b", [128, ET], F32, kind="ExternalInput")

    y_t = nc.dram_tensor("y_t", [E, TOWN], F32R, kind="ExternalOutput")
    if debug:
        k_dbg = nc.dram_tensor("k_dbg", [128, ET, S], BF16,
                               kind="ExternalOutput")
        q_dbg = nc.dram_tensor("q_dbg", [128, ET, TOWN], BF16,
                               kind="ExternalOutput")
        v_dbg = nc.dram_tensor("v_dbg", [128, NT, H, 65], BF16,
                               kind="ExternalOutput")
        ctxn_dbg = nc.dram_tensor("ctxn_dbg", [128, ET, TOWN], BF16,
                                  kind="ExternalOutput")
        x2_dbg = nc.dram_tensor("x2_dbg", [128, ET, TOWN], F32R,
                                kind="ExternalOutput")
        h_dbg = nc.dram_tensor("h_dbg", [128, FT, TOWN], F32R,
                               kind="ExternalOutput")
        s_dbg = nc.dram_tensor("s_dbg", [2, 128, 2, 512], F32,
                               kind="ExternalOutput")
        pr_dbg = nc.dram_tensor("pr_dbg", [2, 128, 2, 512], BF16,
                               kind="ExternalOutput")
        cx_dbg = nc.dram_tensor("cx_dbg", [2, 65, 512], F32,
                                kind="ExternalOutput")
        rec_dbg = nc.dram_tensor("rec_dbg", [2, 512], F32,
                                 kind="ExternalOutput")
        recA_dbg = nc.dram_tensor("recA_dbg", [2, 512], F32,
                                  kind="ExternalOutput")
        rb_dbg = nc.dram_tensor("rb_dbg", [2, 64, 512], F32,
                                kind="ExternalOutput")

    inv_e = 1.0 / E
    unb = float(E) / (E - 1.0)  # E/(E-1) for unbiased variance

    with tile.TileContext(nc) as tc, ExitStack() as ctx:
        consts = ctx.enter_context(tc.tile_pool(name="consts", bufs=1))
        ones_f32 = consts.tile([128, 256], F32)
        nc.vector.memset(ones_f32, 1.0)
        ones128 = consts.tile([128, 128], F32R)
        nc.vector.tensor_copy(ones128, ones_f32[:, 0:128])
        qb_sb = consts.tile([128, ET], F32)
        kb_sb = consts.tile([128, ET], F32)
        ob_sb = consts.tile([128, ET], F32)
        f2b_sb = consts.tile([128, ET], F32)
        f1b_sb = consts.tile([128, FT], F32)
        nc.sync.dma_start(out=qb_sb, in_=qb[:, :])
        nc.sync.dma_start(out=kb_sb, in_=kb[:, :])
        nc.sync.dma_start(out=ob_sb, in_=ob[:, :])
        nc.sync.dma_start(out=f2b_sb, in_=f2b[:, :])
        nc.sync.dma_start(out=f1b_sb, in_=f1b[:, :])
        # v bias broadcast across all partitions (v is token-major)
        vb_bc = consts.tile([128, E], F32)
        with tc.tile_pool(name="vbrow_p", bufs=1) as vbrow_p, \
             tc.tile_pool(name="vbbc_p", bufs=2, space="PSUM") as vbbc_p:
            vb_row = vbrow_p.tile([1, E], F32R)
            nc.sync.dma_start(out=vb_row, in_=vb[None, :])
            for c in range(2):
                ps = vbbc_p.tile([128, 512], F32, tag="vbbc")
                nc.tensor.matmul(ps, ones128[0:1, :],
                                 vb_row[:, c * 512:(c + 1) * 512],
                                 start=True, stop=True)
                nc.scalar.activation(vb_bc[:, c * 512:(c + 1) * 512], ps,
                                     AF.Copy)

        xre = x_t.rearrange("(a p) s -> p a s", p=128)

        # Long-lived cross-phase tensors (strict LIFO pool nesting):
        # z2 (C..F, closed after G) > k/q/v (AB..C) ; h (F..G) ; x2 via DRAM.
        dram = ctx.enter_context(tc.tile_pool(name="dram", bufs=1,
                                              space="DRAM"))
        x2_d = dram.tile([128, ET, TOWN], F32R)
        z2_d = dram.tile([128, ET, TOWN], F32R)

        s_kq = ExitStack()
        pkq = s_kq.enter_context(tc.tile_pool(name="pkq", bufs=1))
        k_sb = pkq.tile([128, ET, S], BF16)          # 32KB/part
        q_sb = pkq.tile([128, ET, TOWN], BF16)       # 16KB/part
        pkv = s_kq.enter_context(tc.tile_pool(name="pkv", bufs=1))
        # [part = t%128, t_tile, head, 64 v dims + 1 ones col]
        v_sb = pkv.tile([128, NT, H, 65], BF16)      # 33KB/part
        nc.vector.tensor_copy(
            v_sb[:, :, :, 64],
            ones_f32[:, 0:NT * H].rearrange("p (a b) -> p a b", a=NT))

        # ================= Phase AB: fused LN1 stats + z1 + QKV ===========
        with tc.tile_pool(name="pab_wv", bufs=1) as pab_wv, \
             tc.tile_pool(name="pab_wk", bufs=4) as pab_wk, \
             tc.tile_pool(name="pab_x", bufs=2) as pab_x, \
             tc.tile_pool(name="pab_sq", bufs=3) as pab_sq, \
             tc.tile_pool(name="pab_ms", bufs=1) as pab_ms, \
             tc.tile_pool(name="pab_st", bufs=2) as pab_st, \
             tc.tile_pool(name="pab_z1", bufs=2) as pab_z1, \
             tc.tile_pool(name="pab_stps", bufs=2, space="PSUM") as pab_stps, \
             tc.tile_pool(name="pab_ps", bufs=4, space="PSUM") as pab_ps:
            wv_sb = pab_wv.tile([128, 2, ET, 512], BF16)   # 16KB/part
            nc.sync.dma_start(out=wv_sb, in_=wv_t[:, :, :, :])

            for c in range(NCH):
                csl = slice(c * CA, (c + 1) * CA)
                xc = pab_x.tile([128, ET, CA], F32R, tag="xc",
                                name=f"xc{c}")
                nc.sync.dma_start(out=xc, in_=xre[:, :, csl])
                # --- stats over features via ones-matmul (broadcast out) ---
                ps_sum = pab_stps.tile([128, CA], F32, tag="sum")
                ps_ssq = pab_stps.tile([128, CA], F32, tag="ssq")
                for a in range(ET):
                    xa = xc[:, a, :]
                    sq = pab_sq.tile([128, CA], F32R, tag="sq")
                    nc.scalar.activation(sq, xa, AF.Square)
                    nc.tensor.matmul(ps_sum, ones128, xa,
                                     start=(a == 0), stop=(a == ET - 1))
                    nc.tensor.matmul(ps_ssq, ones128, sq,
                                     start=(a == 0), stop=(a == ET - 1))
                mean_c = pab_st.tile([128, CA], F32, tag="mean")
                rstd_c = pab_st.tile([128, CA], F32, tag="rstd")
                nc.vector.tensor_scalar_mul(mean_c, ps_sum, inv_e)
                msq = pab_ms.tile([128, CA], F32, tag="msq")
                nc.vector.tensor_tensor(msq, mean_c, mean_c, OP.mult)
                nc.vector.tensor_scalar_mul(msq, msq, unb)
                var = pab_ms.tile([128, CA], F32, tag="var")
                nc.vector.tensor_scalar(var, ps_ssq, 1.0 / (E - 1.0), None,
                                        OP.mult)
                nc.vector.tensor_tensor(var, var, msq, OP.subtract)
                std = pab_ms.tile([128, CA], F32, tag="std")
                nc.scalar.activation(std, var, AF.Sqrt)
                nc.vector.tensor_scalar_add(std, std, EPS)
                nc.vector.reciprocal_approx_fast(out=rstd_c, in_=std)
                # --- z1 chunk ---
                z1c = pab_z1.tile([128, ET, CA], BF16, tag="z1",
                                  name=f"z1c{c}")
                for a in range(ET):
                    nc.vector.tensor_tensor(z1c[:, a, :], xc[:, a, :],
                                            mean_c, OP.subtract)
                    nc.vector.tensor_tensor(z1c[:, a, :], z1c[:, a, :],
                                            rstd_c, OP.mult)
                # --- K proj for this chunk ---
                for ot in range(ET):
                    wk_ot = pab_wk.tile([128, ET, 128], BF16, tag="w",
                                        name=f"wk{c}_{ot}")
                    nc.sync.dma_start(out=wk_ot, in_=wk_t[ot])
                    psk = pab_ps.tile([128, CA], F32, tag="qkv")
                    for a in range(ET):
                        nc.tensor.matmul(psk, wk_ot[:, a, :], z1c[:, a, :],
                                         start=(a == 0), stop=(a == ET - 1))
                    nc.scalar.activation(k_sb[:, ot, csl], psk, AF.Identity,
                                         bias=kb_sb[:, ot:ot + 1])
                # --- V proj (token-major) for this chunk ---
                for tt in range(CA // 128):
                    t0 = tt * 128
                    gt = c * (CA // 128) + tt  # global token tile
                    for vc in range(2):
                        psv = pab_ps.tile([128, 512], F32, tag="qkv")
                        for a in range(ET):
                            nc.tensor.matmul(
                                psv, z1c[:, a, t0:t0 + 128], wv_sb[:, vc, a, :],
                                start=(a == 0), stop=(a == ET - 1))
                        nc.vector.tensor_tensor(
                            v_sb[:, gt, vc * 8:(vc + 1) * 8, 0:64],
                            psv.rearrange("p (h w) -> p h w", w=64),
                            vb_bc[:, vc * 512:(vc + 1) * 512].rearrange(
                                "p (h w) -> p h w", w=64),
                            OP.add)
                # --- Q proj (own tokens only) ---
                if c * CA < TOWN:
                    for ot in range(ET):
                        wq_ot = pab_wk.tile([128, ET, 128], BF16, tag="w",
                                            name=f"wq{c}_{ot}")
                        nc.sync.dma_start(out=wq_ot, in_=wq_t[ot])
                        psq = pab_ps.tile([128, CA], F32, tag="qkv")
                        for a in range(ET):
                            nc.tensor.matmul(psq, wq_ot[:, a, :],
                                             z1c[:, a, :],
                                             start=(a == 0), stop=(a == ET - 1))
                        nc.scalar.activation(q_sb[:, ot, csl], psq,
                                             AF.Identity,
                                             bias=qb_sb[:, ot:ot + 1])

        if debug:
            nc.sync.dma_start(out=k_dbg[:, :, :], in_=k_sb)
            nc.sync.dma_start(out=q_dbg[:, :, :], in_=q_sb)
            nc.sync.dma_start(out=v_dbg[:, :, :, :], in_=v_sb)

        # ============== Phase C/D/E: attention + out-proj + LN2 ===========
        with tc.tile_pool(name="pctxn", bufs=1) as pctxn, \
             tc.tile_pool(name="pc_pr", bufs=2) as pcpr, \
             tc.tile_pool(name="pc_m", bufs=2) as pcm, \
             tc.tile_pool(name="pc_xo", bufs=1) as pcxo, \
             tc.tile_pool(name="pc_x2", bufs=1) as pcx2, \
             tc.tile_pool(name="pc_z2", bufs=1) as pcz2, \
             tc.tile_pool(name="pc_w", bufs=2) as pcw, \
             tc.tile_pool(name="pc_st", bufs=1) as pcst, \
             tc.tile_pool(name="pc_ms", bufs=1) as pcms, \
             tc.tile_pool(name="pc_sq", bufs=2) as pcsq, \
             tc.tile_pool(name="pc_dbg", bufs=1) as pcdbg, \
             tc.tile_pool(name="pc_pss", bufs=2, space="PSUM") as pss_p, \
             tc.tile_pool(name="pc_psx", bufs=2, space="PSUM") as psx_p, \
             tc.tile_pool(name="pc_psm", bufs=2, space="PSUM") as psm_p:
            ctxn = pctxn.tile([128, ET, TOWN], BF16)     # 16KB/part
            for qc in range(2):
                qsl = slice(qc * 512, (qc + 1) * 512)
                # ---- attention, head pairs row-packed on the PE array ----
                for hp in range(ET):
                    ctx_ps = [psx_p.tile([65, 512], F32, tag="ctx",
                                         name=f"ctx{qc}_{hp}_{i}")
                              for i in range(2)]
                    for k2 in range(NT // 2):
                        s_ps = [pss_p.tile([128, 2, 512], F32, tag="s",
                                           name=f"s{qc}_{hp}_{k2}_{i}")
                                for i in range(2)]
                        for j in range(2):
                            kt = 2 * k2 + j
                            ksl = slice(kt * 128, (kt + 1) * 128)
                            # head 2*hp on array rows 0-63, 2*hp+1 on 64-127
                            nc.tensor.matmul(
                                s_ps[j][:, 0, :], k_sb[0:64, hp, ksl],
                                q_sb[0:64, hp, qsl], start=True, stop=True)
                            nc.tensor.matmul(
                                s_ps[j][:, 1, :], k_sb[64:128, hp, ksl],
                                q_sb[64:128, hp, qsl], start=True, stop=True)
                        prs = []
                        for j in range(2):
                            pr = pcpr.tile([128, 2, 512], BF16, tag="pr",
                                           name=f"pr{qc}_{hp}_{k2}_{j}")
                            nc.scalar.activation(pr, s_ps[j], AF.Exp,
                                                 scale=0.125)
                            prs.append(pr)
                            if debug and qc == 0 and hp == 0 and k2 == 0:
                                scp = pcdbg.tile([128, 2, 512], F32,
                                                 tag="scp", name=f"scp{j}")
                                nc.scalar.activation(scp, s_ps[j], AF.Copy)
                                nc.sync.dma_start(out=s_dbg[j], in_=scp)
                                nc.sync.dma_start(out=pr_dbg[j], in_=pr)
                        for j in range(2):
                            kt = 2 * k2 + j
                            for hh in range(2):
                                nc.tensor.matmul(
                                    ctx_ps[hh], v_sb[:, kt, 2 * hp + hh, :],
                                    prs[j][:, hh, :],
                                    start=(kt == 0), stop=(kt == NT - 1))
                    # ---- normalize: ctxn = ctx / den  (den = row 64) ----
                    for hh in range(2):
                        den_sb = pcm.tile([1, 512], F32, tag="den")
                        nc.vector.tensor_copy(den_sb, ctx_ps[hh][64:65, :])
                        rec = pcm.tile([1, 512], F32, tag="rec")
                        nc.vector.reciprocal_approx_fast(out=rec, in_=den_sb)
                        rec2 = pcm.tile([1, 512], F32R, tag="rec2")
                        nc.vector.tensor_copy(rec2, rec)
                        rb_ps = psm_p.tile([128, 512], F32, tag="mm")
                        nc.tensor.matmul(rb_ps[0:64, :], ones128[0:1, 0:64],
                                         rec2, start=True, stop=True)
                        rb = pcm.tile([64, 512], F32, tag="rb")
                        nc.vector.tensor_copy(rb, rb_ps[0:64, :])
                        if debug and qc == 0 and hp == 0:
                            cxc = pcdbg.tile([65, 512], F32, tag="cxc",
                                             name=f"cxc{hh}")
                            nc.scalar.activation(cxc, ctx_ps[hh], AF.Copy)
                            nc.sync.dma_start(out=cx_dbg[hh], in_=cxc)
                            nc.sync.dma_start(out=rec_dbg[hh][None, :],
                                              in_=rec)
                            nc.sync.dma_start(out=rb_dbg[hh], in_=rb)
                        p0 = hh * 64
                        nc.vector.tensor_tensor(
                            ctxn[p0:p0 + 64, hp, qsl],
                            ctx_ps[hh][0:64, :], rb, OP.mult)

                # ---- out-proj + residual for this half ----
                xo = pcxo.tile([128, ET, 512], F32R, tag="xo",
                               name=f"xo{qc}")
                nc.sync.dma_start(out=xo, in_=xre[:, :, qsl])
                x2c = pcx2.tile([128, ET, 512], F32R, tag="x2c",
                                name=f"x2c{qc}")
                for ot in range(ET):
                    w_ot = pcw.tile([128, ET, 128], BF16, tag="w",
                                    name=f"wo{qc}_{ot}")
                    nc.sync.dma_start(out=w_ot, in_=wout_t[ot])
                    pso = psm_p.tile([128, 512], F32, tag="mm")
                    for a in range(ET):
                        nc.tensor.matmul(pso, w_ot[:, a, :],
                                         ctxn[:, a, qsl],
                                         start=(a == 0), stop=(a == ET - 1))
                    ev = pcsq.tile([128, 512], F32, tag="ev")
                    nc.scalar.activation(ev, pso, AF.Identity,
                                         bias=ob_sb[:, ot:ot + 1])
                    nc.vector.tensor_tensor(x2c[:, ot, :], ev,
                                            xo[:, ot, :], OP.add)
                    nc.sync.dma_start(out=x2_d[:, ot, qsl],
                                      in_=x2c[:, ot, :])
                # ---- LN2 stats + z2 for this half ----
                ps_sum = psm_p.tile([128, 512], F32, tag="mm")
                ps_ssq = psm_p.tile([128, 512], F32, tag="mm")
                for a in range(ET):
                    x2a = x2c[:, a, :]
                    sq = pcsq.tile([128, 512], F32R, tag="sq")
                    nc.scalar.activation(sq, x2a, AF.Square)
                    nc.tensor.matmul(ps_sum, ones128, x2a,
                                     start=(a == 0), stop=(a == ET - 1))
                    nc.tensor.matmul(ps_ssq, ones128, sq,
                                     start=(a == 0), stop=(a == ET - 1))
                mean2 = pcst.tile([128, 512], F32, tag="mean")
                rstd2 = pcst.tile([128, 512], F32, tag="rstd")
                nc.vector.tensor_scalar_mul(mean2, ps_sum, inv_e)
                msq = pcms.tile([128, 512], F32, tag="msq")
                nc.vector.tensor_tensor(msq, mean2, mean2, OP.mult)
                nc.vector.tensor_scalar_mul(msq, msq, unb)
                var = pcms.tile([128, 512], F32, tag="var")
                nc.vector.tensor_scalar(var, ps_ssq, 1.0 / (E - 1.0), None,
                                        OP.mult)
                nc.vector.tensor_tensor(var, var, msq, OP.subtract)
                std = pcms.tile([128, 512], F32, tag="std")
                nc.scalar.activation(std, var, AF.Sqrt)
                nc.vector.tensor_scalar_add(std, std, EPS)
                nc.vector.reciprocal_approx_fast(out=rstd2, in_=std)
                z2c = pcz2.tile([128, ET, 512], F32R, tag="z2c",
                                name=f"z2c{qc}")
                for a in range(ET):
                    nc.vector.tensor_tensor(z2c[:, a, :], x2c[:, a, :],
                                            mean2, OP.subtract)
                    nc.vector.tensor_tensor(z2c[:, a, :], z2c[:, a, :],
                                            rstd2, OP.mult)
                    nc.sync.dma_start(out=z2_d[:, a, qsl], in_=z2c[:, a, :])
            if debug:
                nc.sync.dma_start(out=ctxn_dbg[:, :, :], in_=ctxn)
                nc.sync.dma_start(out=x2_dbg[:, :, qsl], in_=x2c)
        s_kq.close()  # k/q/v dead after attention

        # ====== Phase FG: fused MLP — fc1+gelu per 8-ft block, fc2 accum ==
        with tc.tile_pool(name="pfg_z2", bufs=1) as pz2g, \
             tc.tile_pool(name="pfg_x2", bufs=1) as pgx2, \
             tc.tile_pool(name="pfg_h", bufs=1) as phb, \
             tc.tile_pool(name="pfg_w1", bufs=4) as pfw, \
             tc.tile_pool(name="pfg_w2", bufs=4) as pgw, \
             tc.tile_pool(name="pf_ps", bufs=2, space="PSUM") as pfp, \
             tc.tile_pool(name="pg_ps", bufs=2, space="PSUM") as pgp:
            z2g = pz2g.tile([128, ET, TOWN], F32R)   # 32KB/part
            nc.sync.dma_start(out=z2g, in_=z2_d[:, :, :])
            x2g = pgx2.tile([128, ET, TOWN], F32R)   # 32KB/part
            nc.sync.dma_start(out=x2g, in_=x2_d[:, :, :])
            NFB = 4
            for fb in range(NFB):
                h_blk = phb.tile([128, FT // NFB, TOWN], F32R, tag="h",
                                 name=f"hb{fb}")
                for fi in range(FT // NFB):
                    ft = fb * (FT // NFB) + fi
                    w_ft = pfw.tile([128, ET, 128], F32R, tag="w",
                                    name=f"w1_{ft}")
                    nc.sync.dma_start(out=w_ft, in_=wfc1_t[ft])
                    ps = pfp.tile([128, TOWN], F32, tag="ps")
                    for a in range(ET):
                        for half in range(2):
                            hsl = slice(half * 512, (half + 1) * 512)
                            nc.tensor.matmul(ps[:, hsl], w_ft[:, a, :],
                                             z2g[:, a, hsl],
                                             start=(a == 0),
                                             stop=(a == ET - 1))
                    nc.scalar.activation(h_blk[:, fi, :], ps, AF.Gelu,
                                         bias=f1b_sb[:, ft:ft + 1])
                if debug:
                    nc.sync.dma_start(
                        out=h_dbg[:, fb * (FT // NFB):(fb + 1) * (FT // NFB),
                                  :], in_=h_blk)
                for ot in range(ET):
                    w2t = pgw.tile([128, FT // NFB, 128], F32R, tag="w2",
                                   name=f"w2_{fb}_{ot}")
                    nc.sync.dma_start(
                        out=w2t,
                        in_=wfc2_t[ot][:, fb * (FT // NFB):
                                       (fb + 1) * (FT // NFB), :])
                    psg = pgp.tile([128, TOWN], F32, tag="ps")
                    for fi in range(FT // NFB):
                        for half in range(2):
                            hsl = slice(half * 512, (half + 1) * 512)
                            nc.tensor.matmul(psg[:, hsl], w2t[:, fi, :],
                                             h_blk[:, fi, hsl],
                                             start=(fi == 0),
                                             stop=(fi == FT // NFB - 1))
                    # accumulate into the residual in place
                    nc.vector.tensor_tensor(x2g[:, ot, :], x2g[:, ot, :],
                                            psg, OP.add)
            for ot in range(ET):
                nc.vector.tensor_scalar_add(x2g[:, ot, :], x2g[:, ot, :],
                                            f2b_sb[:, ot:ot + 1])
                nc.sync.dma_start(out=y_t[ot * 128:(ot + 1) * 128, :],
                                  in_=x2g[:, ot, :])

    nc.finalize()
    return nc


_NC_CACHE = {}


def _get_nc():
    if "full" not in _NC_CACHE:
        _NC_CACHE["full"] = _build()
    return _NC_CACHE["full"]


def _tile_w(w_t, n_out_tiles, dtype=ml_dtypes.bfloat16):
    # [E_in, O] (in-feature rows) -> [O//128, 128, E_in//128, 128] so each
    # output-tile's weight block is contiguous (multi-KB runs per partition).
    e_in, o = w_t.shape
    arr = w_t.reshape(e_in // 128, 128, n_out_tiles, o // n_out_tiles)
    return np.ascontiguousarray(arr.transpose(2, 1, 0, 3).astype(dtype))


def _tile_w_res(w_t, n_out_tiles, dtype=ml_dtypes.bfloat16):
    # [E_in, O] -> [128(p), O//128(ot), E_in//128(a), O/(nt)] partition-major
    # resident layout: one contiguous 16KB run per partition.
    e_in, o = w_t.shape
    arr = w_t.reshape(e_in // 128, 128, n_out_tiles, o // n_out_tiles)
    return np.ascontiguousarray(arr.transpose(1, 2, 0, 3).astype(dtype))


def _prepare_in_maps(inputs):
    f = np.float32
    x = np.asarray(inputs["x"], f)
    w_qkv = np.asarray(inputs["w_qkv"], np.float64)
    ln1_w = np.asarray(inputs["ln1_w"], np.float64)
    ln1_b = np.asarray(inputs["ln1_b"], np.float64)
    ln2_w = np.asarray(inputs["ln2_w"], np.float64)
    ln2_b = np.asarray(inputs["ln2_b"], np.float64)
    w_fc1 = np.asarray(inputs["w_fc1"], np.float64)

    wqkv_s = (w_qkv * ln1_w[None, :])  # fold LN1 gamma
    qkv_bias = ln1_b @ np.asarray(inputs["w_qkv"], np.float64).T  # [3E]
    wqkv_t = np.ascontiguousarray(wqkv_s.T, f)  # [E, 3E]
    wq_t = _tile_w(wqkv_t[:, 0:E], ET)
    wk_t = _tile_w(wqkv_t[:, E:2 * E], ET)
    wv_t = _tile_w_res(wqkv_t[:, 2 * E:3 * E], 2)  # [128,2,ET,512]
    col = lambda v: np.ascontiguousarray(
        np.asarray(v, f).reshape(-1, 128).T)  # [o] -> [128, o//128]
    qb = col(qkv_bias[0:E])
    kb = col(qkv_bias[E:2 * E])
    vb = np.ascontiguousarray(qkv_bias[2 * E:3 * E], f)

    wout_t = _tile_w(np.ascontiguousarray(np.asarray(inputs["w_out"], f).T),
                     ET)
    ob = col(inputs["b_out"])

    wfc1_s = (w_fc1 * ln2_w[None, :])
    f1b_flat = np.asarray(inputs["b_fc1"], np.float64) + ln2_b @ w_fc1.T
    f1b = col(f1b_flat)
    wfc1_t = _tile_w(np.ascontiguousarray(wfc1_s.T, f), FT, np.float32)
    wfc2_t = _tile_w(np.ascontiguousarray(np.asarray(inputs["w_fc2"], f).T),
                     ET, np.float32)
    f2b = col(inputs["b_fc2"])

    shared = dict(wq_t=wq_t, wk_t=wk_t, wv_t=wv_t, qb=qb, kb=kb, vb=vb,
                  wout_t=wout_t, ob=ob, wfc1_t=wfc1_t, f1b=f1b,
                  wfc2_t=wfc2_t, f2b=f2b)
    in_maps = []
    for core in range(NCORES):
        b, hf = divmod(core, 2)
        xs = np.roll(x[b], -hf * TOWN, axis=0)  # own tokens first
        x_tc = np.ascontiguousarray(xs.T)  # [E, S]
        in_maps.append(dict(x_t=x_tc, **shared))
    return in_maps


def _assemble(inputs, results):
    f = np.float32
    out = np.empty((B, S, E), f)
    for core in range(NCORES):
        b, hf = divmod(core, 2)
        out[b, hf * TOWN:(hf + 1) * TOWN, :] = results[core]["y_t"].T
    return out


def run(inputs, **spmd_kwargs):
    nc = _get_nc()
    in_maps = _prepare_in_maps(inputs)
    res = run_bass_kernel_spmd(nc, in_maps, core_ids=list(range(NCORES)),
                               **spmd_kwargs)
    return _assemble(inputs, res.results), res


def kernel(**inputs):
    out, _ = run(inputs)
    return out


# revision 20
# speedup vs baseline: 1.0591x; 1.0591x over previous
"""Encoder layer (pre-norm attention + MLP) on 8 Trainium2 cores.

Sharding: core = (batch b in 0..3, half hf in 0..1). Each core receives the
full 2048-token sequence of batch b, transposed to [E, S] and rolled so the
core's own 1024 tokens are columns 0:1024 (attention and LN are invariant to
key order, so rolling keeps the program identical across cores). The core
computes K/V over the full sequence and everything else only for its own
tokens. No collectives; the host reassembles the 8 shards.

Structure (v5):
- Phase AB: LN1 stats + z1 + QKV fused per 512-token chunk so the PE never
  waits on the full-sequence layernorm. K/Q/V land in SBUF (bf16).
- Phase C: attention. Score matmuls (contraction=64) are row-packed in head
  pairs on the two 64-row halves of the PE array; exp is batched over 2-bank
  PSUM tiles and is the only ACT work in this phase (the phase is exp-bound).
  ctx+denominator are evicted to SBUF immediately to recycle PSUM banks; the
  softmax denominator uses the fast approximate DVE reciprocal; out-proj and
  LN2 *stats* for each 512-token half interleave with the next half's
  attention (evictions on DVE, stats to DRAM, no ACT contention).
- Phase FG: fused MLP in full f32r — z2 is recomputed from the reloaded
  residual (so it is never stored in bf16), fc1+gelu produce one 8-ft block
  of h at a time (f32r, SBUF-transient), which fc2 immediately consumes,
  accumulating into the residual in place. No bf16 anywhere in the MLP.
"""

import numpy as np
import ml_dtypes
from contextlib import ExitStack

import concourse.bacc as bacc
import concourse.mybir as mybir
import concourse.tile as tile
from concourse.bass_utils import run_bass_kernel_spmd

F32 = mybir.dt.float32
F32R = mybir.dt.float32r
BF16 = mybir.dt.bfloat16
AF = mybir.ActivationFunctionType
OP = mybir.AluOpType

B, S, E, H, D, FF = 4, 2048, 1024, 16, 64, 4096
TOWN = 1024  # tokens owned per core
ET = E // 128  # 8
FT = FF // 128  # 32
NT = S // 128  # 16 token tiles (full seq)
NCORES = 8
EPS = 1e-6
CA = 512  # token chunk for the fused LN1+QKV phase
NCH = S // CA  # 4
NFB = 4  # ft blocks in the fused MLP


def _build(debug=False):
    nc = bacc.Bacc()

    x_t = nc.dram_tensor("x_t", [E, S], F32R, kind="ExternalInput")
    # streamed per-out-tile qkv weights: [out_tile, 128(p), in_tile, 128]
    wq_t = nc.dram_tensor("wq_t", [ET, 128, ET, 128], BF16, kind="ExternalInput")
    wk_t = nc.dram_tensor("wk_t", [ET, 128, ET, 128], BF16, kind="ExternalInput")
    # resident v weights, partition-major: [128(p), half, in_tile, 512]
    wv_t = nc.dram_tensor("wv_t", [128, 2, ET, 512], BF16, kind="ExternalInput")
    qb = nc.dram_tensor("qb", [128, ET], F32, kind="ExternalInput")
    kb = nc.dram_tensor("kb", [128, ET], F32, kind="ExternalInput")
    vb = nc.dram_tensor("vb", [E], F32R, kind="ExternalInput")
    wout_t = nc.dram_tensor("wout_t", [ET, 128, ET, 128], BF16,
                            kind="ExternalInput")
    ob = nc.dram_tensor("ob", [128, ET], F32, kind="ExternalInput")
    wfc1_t = nc.dram_tensor("wfc1_t", [FT, 128, ET, 128], F32R,
                            kind="ExternalInput")
    f1b = nc.dram_tensor("f1b", [128, FT], F32, kind="ExternalInput")
    wfc2_t = nc.dram_tensor("wfc2_t", [ET, 128, FT, 128], F32R,
                            kind="ExternalInput")
    f2b = nc.dram_tensor("f2b", [128, ET], F32, kind="ExternalInput")

    y_t = nc.dram_tensor("y_t", [E, TOWN], F32R, kind="ExternalOutput")
    if debug:
        k_dbg = nc.dram_tensor("k_dbg", [128, ET, S], BF16,
                               kind="ExternalOutput")
        q_dbg = nc.dram_tensor("q_dbg", [128, ET, TOWN], BF16,
                               kind="ExternalOutput")
        v_dbg = nc.dram_tensor("v_dbg", [128, NT, H, 65], BF16,
                               kind="ExternalOutput")
        ctxn_dbg = nc.dram_tensor("ctxn_dbg", [128, ET, TOWN], BF16,
                                  kind="ExternalOutput")
        x2_dbg = nc.dram_tensor("x2_dbg", [128, ET, TOWN], F32R,
                                kind="ExternalOutput")
        h_dbg = nc.dram_tensor("h_dbg", [128, FT, TOWN], F32R,
                               kind="ExternalOutput")
        s_dbg = nc.dram_tensor("s_dbg", [2, 128, 2, 512], F32,
                               kind="ExternalOutput")
        pr_dbg = nc.dram_tensor("pr_dbg", [2, 128, 2, 512], BF16,
                                kind="ExternalOutput")
        cx_dbg = nc.dram_tensor("cx_dbg", [2, 65, 512], F32,
                                kind="ExternalOutput")
        rec_dbg = nc.dram_tensor("rec_dbg", [2, 512], F32,
                                 kind="ExternalOutput")
        rb_dbg = nc.dram_tensor("rb_dbg", [2, 64, 512], F32,
                                kind="ExternalOutput")

    inv_e = 1.0 / E
    unb = float(E) / (E - 1.0)  # E/(E-1) for unbiased variance

    with tile.TileContext(nc) as tc, ExitStack() as ctx:
        consts = ctx.enter_context(tc.tile_pool(name="consts", bufs=1))
        ones_f32 = consts.tile([128, 256], F32)
        nc.vector.memset(ones_f32, 1.0)
        ones128 = consts.tile([128, 128], F32R)
        nc.vector.tensor_copy(ones128, ones_f32[:, 0:128])
        qb_sb = consts.tile([128, ET], F32)
        kb_sb = consts.tile([128, ET], F32)
        ob_sb = consts.tile([128, ET], F32)
        f2b_sb = consts.tile([128, ET], F32)
        f1b_sb = consts.tile([128, FT], F32)
        nc.sync.dma_start(out=qb_sb, in_=qb[:, :])
        nc.sync.dma_start(out=kb_sb, in_=kb[:, :])
        nc.sync.dma_start(out=ob_sb, in_=ob[:, :])
        nc.sync.dma_start(out=f2b_sb, in_=f2b[:, :])
        nc.sync.dma_start(out=f1b_sb, in_=f1b[:, :])
        # v bias broadcast across all partitions (v is token-major)
        vb_bc = consts.tile([128, E], F32)
        with tc.tile_pool(name="vbrow_p", bufs=1) as vbrow_p, \
             tc.tile_pool(name="vbbc_p", bufs=2, space="PSUM") as vbbc_p:
            vb_row = vbrow_p.tile([1, E], F32R)
            nc.sync.dma_start(out=vb_row, in_=vb[None, :])
            for c in range(2):
                ps = vbbc_p.tile([128, 512], F32, tag="vbbc")
                nc.tensor.matmul(ps, ones128[0:1, :],
                                 vb_row[:, c * 512:(c + 1) * 512],
                                 start=True, stop=True)
                nc.scalar.activation(vb_bc[:, c * 512:(c + 1) * 512], ps,
                                     AF.Copy)

        xre = x_t.rearrange("(a p) s -> p a s", p=128)

        dram = ctx.enter_context(tc.tile_pool(name="dram", bufs=1,
                                              space="DRAM"))
        x2_d = dram.tile([128, ET, TOWN], F32R)
        mv_d = dram.tile([128, 2, 2, 512], F32)  # LN2 mean/var per half

        # fc1 weight stream pool opened early so the first tiles can
        # prefetch long before the MLP phase starts.
        s_w1 = ExitStack()
        pfw = s_w1.enter_context(tc.tile_pool(name="pfg_w1", bufs=4))

        s_kq = ExitStack()
        pkq = s_kq.enter_context(tc.tile_pool(name="pkq", bufs=1))
        k_sb = pkq.tile([128, ET, S], BF16)          # 32KB/part
        q_sb = pkq.tile([128, ET, TOWN], BF16)       # 16KB/part
        pkv = s_kq.enter_context(tc.tile_pool(name="pkv", bufs=1))
        # [part = t%128, t_tile, head, 64 v dims + 1 ones col]
        v_sb = pkv.tile([128, NT, H, 65], BF16)      # 33KB/part
        nc.vector.tensor_copy(
            v_sb[:, :, :, 64],
            ones_f32[:, 0:NT * H].rearrange("p (a b) -> p a b", a=NT))

        # ================= Phase AB: fused LN1 stats + z1 + QKV ===========
        with tc.tile_pool(name="pab_wv", bufs=1) as pab_wv, \
             tc.tile_pool(name="pab_wk", bufs=4) as pab_wk, \
             tc.tile_pool(name="pab_x", bufs=2) as pab_x, \
             tc.tile_pool(name="pab_sq", bufs=3) as pab_sq, \
             tc.tile_pool(name="pab_ms", bufs=1) as pab_ms, \
             tc.tile_pool(name="pab_st", bufs=2) as pab_st, \
             tc.tile_pool(name="pab_z1", bufs=2) as pab_z1, \
             tc.tile_pool(name="pab_stps", bufs=2, space="PSUM") as pab_stps, \
             tc.tile_pool(name="pab_ps", bufs=4, space="PSUM") as pab_ps:
            wv_sb = pab_wv.tile([128, 2, ET, 512], BF16)   # 16KB/part
            nc.sync.dma_start(out=wv_sb, in_=wv_t[:, :, :, :])

            for c in range(NCH):
                csl = slice(c * CA, (c + 1) * CA)
                xc = pab_x.tile([128, ET, CA], F32R, tag="xc",
                                name=f"xc{c}")
                for a in range(ET):
                    nc.sync.dma_start(out=xc[:, a, :], in_=xre[:, a, csl])
                # --- stats over features via ones-matmul (broadcast out) ---
                ps_sum = pab_stps.tile([128, CA], F32, tag="sum")
                ps_ssq = pab_stps.tile([128, CA], F32, tag="ssq")
                for a in range(ET):
                    xa = xc[:, a, :]
                    sq = pab_sq.tile([128, CA], F32R, tag="sq")
                    nc.scalar.activation(sq, xa, AF.Square)
                    nc.tensor.matmul(ps_sum, ones128, xa,
                                     start=(a == 0), stop=(a == ET - 1))
                    nc.tensor.matmul(ps_ssq, ones128, sq,
                                     start=(a == 0), stop=(a == ET - 1))
                mean_c = pab_st.tile([128, CA], F32, tag="mean")
                rstd_c = pab_st.tile([128, CA], F32, tag="rstd")
                nc.vector.tensor_scalar_mul(mean_c, ps_sum, inv_e)
                msq = pab_ms.tile([128, CA], F32, tag="msq")
                nc.vector.tensor_tensor(msq, mean_c, mean_c, OP.mult)
                nc.vector.tensor_scalar_mul(msq, msq, unb)
                var = pab_ms.tile([128, CA], F32, tag="var")
                nc.vector.tensor_scalar(var, ps_ssq, 1.0 / (E - 1.0), None,
                                        OP.mult)
                nc.vector.tensor_tensor(var, var, msq, OP.subtract)
                std = pab_ms.tile([128, CA], F32, tag="std")
                nc.scalar.activation(std, var, AF.Sqrt)
                nc.vector.tensor_scalar_add(std, std, EPS)
                nc.vector.reciprocal_approx_fast(out=rstd_c, in_=std)
                # --- z1 chunk ---
                z1c = pab_z1.tile([128, ET, CA], BF16, tag="z1",
                                  name=f"z1c{c}")
                for a in range(ET):
                    nc.vector.tensor_tensor(z1c[:, a, :], xc[:, a, :],
                                            mean_c, OP.subtract)
                    nc.vector.tensor_tensor(z1c[:, a, :], z1c[:, a, :],
                                            rstd_c, OP.mult)
                # --- K proj for this chunk ---
                for ot in range(ET):
                    wk_ot = pab_wk.tile([128, ET, 128], BF16, tag="w",
                                        name=f"wk{c}_{ot}")
                    nc.sync.dma_start(out=wk_ot, in_=wk_t[ot])
                    psk = pab_ps.tile([128, CA], F32, tag="qkv")
                    for a in range(ET):
                        nc.tensor.matmul(psk, wk_ot[:, a, :], z1c[:, a, :],
                                         start=(a == 0), stop=(a == ET - 1))
                    nc.scalar.activation(k_sb[:, ot, csl], psk, AF.Identity,
                                         bias=kb_sb[:, ot:ot + 1])
                # --- V proj (token-major) for this chunk ---
                for tt in range(CA // 128):
                    t0 = tt * 128
                    gt = c * (CA // 128) + tt  # global token tile
                    for vc in range(2):
                        psv = pab_ps.tile([128, 512], F32, tag="qkv")
                        for a in range(ET):
                            nc.tensor.matmul(
                                psv, z1c[:, a, t0:t0 + 128], wv_sb[:, vc, a, :],
                                start=(a == 0), stop=(a == ET - 1))
                        nc.vector.tensor_tensor(
                            v_sb[:, gt, vc * 8:(vc + 1) * 8, 0:64],
                            psv.rearrange("p (h w) -> p h w", w=64),
                            vb_bc[:, vc * 512:(vc + 1) * 512].rearrange(
                                "p (h w) -> p h w", w=64),
                            OP.add)
                # --- Q proj (own tokens only) ---
                if c * CA < TOWN:
                    for ot in range(ET):
                        wq_ot = pab_wk.tile([128, ET, 128], BF16, tag="w",
                                            name=f"wq{c}_{ot}")
                        nc.sync.dma_start(out=wq_ot, in_=wq_t[ot])
                        psq = pab_ps.tile([128, CA], F32, tag="qkv")
                        for a in range(ET):
                            nc.tensor.matmul(psq, wq_ot[:, a, :],
                                             z1c[:, a, :],
                                             start=(a == 0), stop=(a == ET - 1))
                        nc.scalar.activation(q_sb[:, ot, csl], psq,
                                             AF.Identity,
                                             bias=qb_sb[:, ot:ot + 1])

        if debug:
            nc.sync.dma_start(out=k_dbg[:, :, :], in_=k_sb)
            nc.sync.dma_start(out=q_dbg[:, :, :], in_=q_sb)
            nc.sync.dma_start(out=v_dbg[:, :, :, :], in_=v_sb)

        # ============== Phase C: attention + out-proj + LN2 stats =========
        with tc.tile_pool(name="pctxn", bufs=1) as pctxn, \
             tc.tile_pool(name="pc_pr", bufs=2) as pcpr, \
             tc.tile_pool(name="pc_cx", bufs=4) as pccx, \
             tc.tile_pool(name="pc_m", bufs=2) as pcm, \
             tc.tile_pool(name="pc_xo", bufs=1) as pcxo, \
             tc.tile_pool(name="pc_x2", bufs=1) as pcx2, \
             tc.tile_pool(name="pc_w", bufs=2) as pcw, \
             tc.tile_pool(name="pc_st", bufs=1) as pcst, \
             tc.tile_pool(name="pc_ms", bufs=1) as pcms, \
             tc.tile_pool(name="pc_sq", bufs=2) as pcsq, \
             tc.tile_pool(name="pc_dbg", bufs=1) as pcdbg, \
             tc.tile_pool(name="pc_pss", bufs=2, space="PSUM") as pss_p, \
             tc.tile_pool(name="pc_psx", bufs=2, space="PSUM") as psx_p, \
             tc.tile_pool(name="pc_psm", bufs=2, space="PSUM") as psm_p:
            ctxn = pctxn.tile([128, ET, TOWN], BF16)     # 16KB/part
            for qc in range(2):
                qsl = slice(qc * 512, (qc + 1) * 512)
                # ---- attention, head pairs row-packed on the PE array ----
                for hp in range(ET):
                    ctx_ps = [psx_p.tile([65, 512], F32, tag="ctx",
                                         name=f"ctx{qc}_{hp}_{i}")
                              for i in range(2)]
                    for k2 in range(NT // 2):
                        s_ps = [pss_p.tile([128, 2, 512], F32, tag="s",
                                           name=f"s{qc}_{hp}_{k2}_{i}")
                                for i in range(2)]
                        for j in range(2):
                            kt = 2 * k2 + j
                            ksl = slice(kt * 128, (kt + 1) * 128)
                            # head 2*hp on array rows 0-63, 2*hp+1 on 64-127
                            nc.tensor.matmul(
                                s_ps[j][:, 0, :], k_sb[0:64, hp, ksl],
                                q_sb[0:64, hp, qsl], start=True, stop=True)
                            nc.tensor.matmul(
                                s_ps[j][:, 1, :], k_sb[64:128, hp, ksl],
                                q_sb[64:128, hp, qsl], start=True, stop=True)
                        prs = []
                        for j in range(2):
                            pr = pcpr.tile([128, 2, 512], BF16, tag="pr",
                                           name=f"pr{qc}_{hp}_{k2}_{j}")
                            nc.scalar.activation(pr, s_ps[j], AF.Exp,
                                                 scale=0.125)
                            prs.append(pr)
                            if debug and qc == 0 and hp == 0 and k2 == 0:
                                scp = pcdbg.tile([128, 2, 512], F32,
                                                 tag="scp", name=f"scp{j}")
                                nc.scalar.activation(scp, s_ps[j], AF.Copy)
                                nc.sync.dma_start(out=s_dbg[j], in_=scp)
                                nc.sync.dma_start(out=pr_dbg[j], in_=pr)
                        for j in range(2):
                            kt = 2 * k2 + j
                            for hh in range(2):
                                nc.tensor.matmul(
                                    ctx_ps[hh], v_sb[:, kt, 2 * hp + hh, :],
                                    prs[j][:, hh, :],
                                    start=(kt == 0), stop=(kt == NT - 1))
                    # ---- evict ctx+den to SBUF fast, normalize there ----
                    for hh in range(2):
                        cxs = pccx.tile([65, 512], F32, tag="cxs")
                        nc.vector.tensor_copy(cxs, ctx_ps[hh])
                        # approx recip needs a partition-0 input base
                        den_sb = pcm.tile([1, 512], F32, tag="den")
                        nc.vector.tensor_copy(den_sb, ctx_ps[hh][64:65, :])
                        rec = pcm.tile([1, 512], F32, tag="rec")
                        nc.vector.reciprocal_approx_fast(
                            out=rec, in_=den_sb)
                        rec2 = pcm.tile([1, 512], F32R, tag="rec2")
                        nc.vector.tensor_copy(rec2, rec)
                        rb_ps = psm_p.tile([128, 512], F32, tag="mm")
                        nc.tensor.matmul(rb_ps[0:64, :], ones128[0:1, 0:64],
                                         rec2, start=True, stop=True)
                        rb = pcm.tile([64, 512], F32, tag="rb")
                        nc.vector.tensor_copy(rb, rb_ps[0:64, :])
                        p0 = hh * 64
                        nc.vector.tensor_tensor(
                            ctxn[p0:p0 + 64, hp, qsl],
                            cxs[0:64, :], rb, OP.mult)
                        if debug and qc == 0 and hp == 0:
                            nc.sync.dma_start(out=cx_dbg[hh], in_=cxs)
                            nc.sync.dma_start(out=rec_dbg[hh][None, :],
                                              in_=rec)
                            nc.sync.dma_start(out=rb_dbg[hh], in_=rb)

                # ---- out-proj + residual for this half ----
                xo = pcxo.tile([128, ET, 512], F32R, tag="xo",
                               name=f"xo{qc}")
                nc.sync.dma_start(out=xo, in_=xre[:, :, qsl])
                x2c = pcx2.tile([128, ET, 512], F32R, tag="x2c",
                                name=f"x2c{qc}")
                for ot in range(ET):
                    w_ot = pcw.tile([128, ET, 128], BF16, tag="w",
                                    name=f"wo{qc}_{ot}")
                    nc.sync.dma_start(out=w_ot, in_=wout_t[ot])
                    pso = psm_p.tile([128, 512], F32, tag="mm")
                    for a in range(ET):
                        nc.tensor.matmul(pso, w_ot[:, a, :],
                                         ctxn[:, a, qsl],
                                         start=(a == 0), stop=(a == ET - 1))
                    ev = pcsq.tile([128, 512], F32, tag="ev")
                    nc.vector.tensor_scalar_add(ev, pso, ob_sb[:, ot:ot + 1])
                    nc.vector.tensor_tensor(x2c[:, ot, :], ev,
                                            xo[:, ot, :], OP.add)
                    nc.sync.dma_start(out=x2_d[:, ot, qsl],
                                      in_=x2c[:, ot, :])
                # ---- LN2 stats for this half (sqrt/z2 deferred to FG) ----
                ps_sum = psm_p.tile([128, 512], F32, tag="mm")
                ps_ssq = psm_p.tile([128, 512], F32, tag="mm")
                for a in range(ET):
                    x2a = x2c[:, a, :]
                    sq = pcsq.tile([128, 512], F32R, tag="sq")
                    nc.vector.tensor_tensor(sq, x2a, x2a, OP.mult)
                    nc.tensor.matmul(ps_sum, ones128, x2a,
                                     start=(a == 0), stop=(a == ET - 1))
                    nc.tensor.matmul(ps_ssq, ones128, sq,
                                     start=(a == 0), stop=(a == ET - 1))
                mean2 = pcst.tile([128, 512], F32, tag="mean")
                nc.vector.tensor_scalar_mul(mean2, ps_sum, inv_e)
                nc.sync.dma_start(out=mv_d[:, qc, 0, :], in_=mean2)
                msq = pcms.tile([128, 512], F32, tag="msq")
                nc.vector.tensor_tensor(msq, mean2, mean2, OP.mult)
                nc.vector.tensor_scalar_mul(msq, msq, unb)
                var = pcms.tile([128, 512], F32, tag="var")
                nc.vector.tensor_scalar(var, ps_ssq, 1.0 / (E - 1.0), None,
                                        OP.mult)
                nc.vector.tensor_tensor(var, var, msq, OP.subtract)
                nc.sync.dma_start(out=mv_d[:, qc, 1, :], in_=var)
            if debug:
                nc.sync.dma_start(out=ctxn_dbg[:, :, :], in_=ctxn)
        s_kq.close()  # k/q/v dead after attention

        # ====== Phase FG: fused MLP — fc1+gelu per 8-ft block, fc2 accum ==
        with tc.tile_pool(name="pfg_x2", bufs=1) as pgx2, \
             tc.tile_pool(name="pfg_z2", bufs=1) as pz2g, \
             tc.tile_pool(name="pfg_mv", bufs=1) as pmv, \
             tc.tile_pool(name="pfg_h", bufs=2) as phb, \
             tc.tile_pool(name="pfg_w2", bufs=4) as pgw, \
             tc.tile_pool(name="pf_ps", bufs=2, space="PSUM") as pfp, \
             tc.tile_pool(name="pg_ps", bufs=2, space="PSUM") as pgp:
            x2g = pgx2.tile([128, ET, TOWN], F32R)   # 32KB/part
            z2g = pz2g.tile([128, ET, TOWN], F32R)   # 32KB/part
            mv_sb = pmv.tile([128, 2, 2, 512], F32)
            rstd2 = pmv.tile([128, 2, 512], F32)
            std2 = pmv.tile([128, 2, 512], F32)
            nc.sync.dma_start(out=mv_sb, in_=mv_d[:, :, :, :])
            nc.scalar.activation(std2[:, 0, :], mv_sb[:, 0, 1, :], AF.Sqrt)
            nc.scalar.activation(std2[:, 1, :], mv_sb[:, 1, 1, :], AF.Sqrt)
            nc.vector.tensor_scalar_add(std2, std2, EPS)
            nc.vector.reciprocal_approx_fast(out=rstd2, in_=std2)
            for qc in range(2):
                qsl = slice(qc * 512, (qc + 1) * 512)
                for a in range(ET):
                    nc.sync.dma_start(out=x2g[:, a, qsl],
                                      in_=x2_d[:, a, qsl])
                    nc.vector.tensor_tensor(z2g[:, a, qsl], x2g[:, a, qsl],
                                            mv_sb[:, qc, 0, :], OP.subtract)
                    nc.vector.tensor_tensor(z2g[:, a, qsl], z2g[:, a, qsl],
                                            rstd2[:, qc, :], OP.mult)
            NFT = FT // NFB
            for fb in range(NFB):
                h_blk = phb.tile([128, NFT, TOWN], F32R, tag="h",
                                 name=f"hb{fb}")
                for fi in range(NFT):
                    ft = fb * NFT + fi
                    w_ft = pfw.tile([128, ET, 128], F32R, tag="w",
                                    name=f"w1_{ft}")
                    nc.sync.dma_start(out=w_ft, in_=wfc1_t[ft])
                    ps = pfp.tile([128, TOWN], F32, tag="ps")
                    for half in range(2):
                        hsl = slice(half * 512, (half + 1) * 512)
                        for a in range(ET):
                            nc.tensor.matmul(ps[:, hsl], w_ft[:, a, :],
                                             z2g[:, a, hsl],
                                             start=(a == 0),
                                             stop=(a == ET - 1))
                    nc.scalar.activation(h_blk[:, fi, :], ps, AF.Gelu,
                                         bias=f1b_sb[:, ft:ft + 1])
                if debug:
                    nc.sync.dma_start(
                        out=h_dbg[:, fb * NFT:(fb + 1) * NFT, :], in_=h_blk)
                for ot in range(ET):
                    w2t = pgw.tile([128, NFT, 128], F32R, tag="w2",
                                   name=f"w2_{fb}_{ot}")
                    nc.sync.dma_start(
                        out=w2t,
                        in_=wfc2_t[ot][:, fb * NFT:(fb + 1) * NFT, :])
                    psg = pgp.tile([128, TOWN], F32, tag="ps")
                    for fi in range(NFT):
                        for half in range(2):
                            hsl = slice(half * 512, (half + 1) * 512)
                            nc.tensor.matmul(psg[:, hsl], w2t[:, fi, :],
                                             h_blk[:, fi, hsl],
                                             start=(fi == 0),
                                             stop=(fi == NFT - 1))
                    # accumulate into the residual in place
                    nc.vector.tensor_tensor(x2g[:, ot, :], x2g[:, ot, :],
                                            psg, OP.add)
            for ot in range(ET):
                nc.vector.tensor_scalar_add(x2g[:, ot, :], x2g[:, ot, :],
                                            f2b_sb[:, ot:ot + 1])
                nc.sync.dma_start(out=y_t[ot * 128:(ot + 1) * 128, :],
                                  in_=x2g[:, ot, :])
            if debug:
                nc.sync.dma_start(out=x2_dbg[:, :, :], in_=x2g)
        s_w1.close()

    nc.finalize()
    return nc


_NC_CACHE = {}


def _get_nc():
    if "full" not in _NC_CACHE:
        _NC_CACHE["full"] = _build()
    return _NC_CACHE["full"]


def _tile_w(w_t, n_out_tiles, dtype=ml_dtypes.bfloat16):
    # [E_in, O] (in-feature rows) -> [O//128, 128, E_in//128, 128] so each
    # output-tile's weight block is contiguous (multi-KB runs per partition).
    e_in, o = w_t.shape
    arr = w_t.reshape(e_in // 128, 128, n_out_tiles, o // n_out_tiles)
    return np.ascontiguousarray(arr.transpose(2, 1, 0, 3).astype(dtype))


def _tile_w_res(w_t, n_out_tiles, dtype=ml_dtypes.bfloat16):
    # [E_in, O] -> [128(p), O//128(ot), E_in//128(a), O/(nt)] partition-major
    # resident layout: one contiguous run per partition.
    e_in, o = w_t.shape
    arr = w_t.reshape(e_in // 128, 128, n_out_tiles, o // n_out_tiles)
    return np.ascontiguousarray(arr.transpose(1, 2, 0, 3).astype(dtype))


def _prepare_in_maps(inputs):
    f = np.float32
    x = np.asarray(inputs["x"], f)
    w_qkv = np.asarray(inputs["w_qkv"], np.float64)
    ln1_w = np.asarray(inputs["ln1_w"], np.float64)
    ln1_b = np.asarray(inputs["ln1_b"], np.float64)
    ln2_w = np.asarray(inputs["ln2_w"], np.float64)
    ln2_b = np.asarray(inputs["ln2_b"], np.float64)
    w_fc1 = np.asarray(inputs["w_fc1"], np.float64)

    wqkv_s = (w_qkv * ln1_w[None, :])  # fold LN1 gamma
    qkv_bias = ln1_b @ np.asarray(inputs["w_qkv"], np.float64).T  # [3E]
    wqkv_t = np.ascontiguousarray(wqkv_s.T, f)  # [E, 3E]
    wq_t = _tile_w(wqkv_t[:, 0:E], ET)
    wk_t = _tile_w(wqkv_t[:, E:2 * E], ET)
    wv_t = _tile_w_res(wqkv_t[:, 2 * E:3 * E], 2)  # [128,2,ET,512]
    col = lambda v: np.ascontiguousarray(
        np.asarray(v, f).reshape(-1, 128).T)  # [o] -> [128, o//128]
    qb = col(qkv_bias[0:E])
    kb = col(qkv_bias[E:2 * E])
    vb = np.ascontiguousarray(qkv_bias[2 * E:3 * E], f)

    wout_t = _tile_w(np.ascontiguousarray(np.asarray(inputs["w_out"], f).T),
                     ET)
    ob = col(inputs["b_out"])

    wfc1_s = (w_fc1 * ln2_w[None, :])
    f1b_flat = np.asarray(inputs["b_fc1"], np.float64) + ln2_b @ w_fc1.T
    f1b = col(f1b_flat)
    wfc1_t = _tile_w(np.ascontiguousarray(wfc1_s.T, f), FT, np.float32)
    wfc2_t = _tile_w(np.ascontiguousarray(np.asarray(inputs["w_fc2"], f).T),
                     ET, np.float32)
    f2b = col(inputs["b_fc2"])

    shared = dict(wq_t=wq_t, wk_t=wk_t, wv_t=wv_t, qb=qb, kb=kb, vb=vb,
                  wout_t=wout_t, ob=ob, wfc1_t=wfc1_t, f1b=f1b,
                  wfc2_t=wfc2_t, f2b=f2b)
    in_maps = []
    for core in range(NCORES):
        b, hf = divmod(core, 2)
        xs = np.roll(x[b], -hf * TOWN, axis=0)  # own tokens first
        x_tc = np.ascontiguousarray(xs.T)  # [E, S]
        in_maps.append(dict(x_t=x_tc, **shared))
    return in_maps


def _assemble(inputs, results):
    f = np.float32
    out = np.empty((B, S, E), f)
    for core in range(NCORES):
        b, hf = divmod(core, 2)
        out[b, hf * TOWN:(hf + 1) * TOWN, :] = results[core]["y_t"].T
    return out


def run(inputs, **spmd_kwargs):
    nc = _get_nc()
    in_maps = _prepare_in_maps(inputs)
    res = run_bass_kernel_spmd(nc, in_maps, core_ids=list(range(NCORES)),
                               **spmd_kwargs)
    return _assemble(inputs, res.results), res


def kernel(**inputs):
    out, _ = run(inputs)
    return out
